# revision 18
# baseline (speedup 1.0000x reference)
"""Trainium2 Bass kernel for batched multi-head attention.

Problem: B=8, H=8, S=2048, D=64 f32 attention,
  out = softmax(Q K^T / 64**0.25) V  per (b, h).

Sharding: the 64 (b,h) pairs are split 8-per-core across the 8 NeuronCores
(pure data/head parallelism, no collectives).

Per-core algorithm (per head), everything in the k-partitioned orientation so
no large on-chip transposes are needed:
  - Host pre-transposes Q, K to [D, S] (d-major) and casts Q/K/V to bf16.
  - scoresT[k, q] = K^T.T @ Q^T in k-chunks of 128 x q-slabs of 512.  Q^T/K^T
    are duplicated into partitions 64..127 so chunk pairs run as two K=64
    matmuls packed into disjoint 64-row strips of the PE array (2x).
  - exp on the Scalar engine straight out of PSUM with the 1/64**0.25 scale
    folded in (no max subtraction: |scores/tau| <= ~20, safe in f32).
  - AV keeps expT as the *moving* operand (fast path through the PE) with
    V as the stationary, augmented with a ones column so the softmax
    denominators fall out of the same accumulation: PSUM outT[0:64, q]
    unnormalized, outT[64, q] = sum.  Output stays d-major.
  - Softmax denominators cross partitions via a DRAM bounce (store row,
    reload 128-partition-tiled), reciprocal on DVE, broadcast back with a
    stride-0 DMA, one tensor_tensor multiply normalizes.
  - Host transposes the [D, S] outputs back to [S, D] (free).
"""
import sys

sys.path.insert(0, "/opt/trn_rl_repo")

from contextlib import ExitStack

import ml_dtypes
import numpy as np

import concourse.bass as bass
import concourse.tile as tile
from concourse import bacc, mybir
from concourse.bass_utils import run_bass_kernel_spmd

B, H, S, D = 8, 8, 2048, 64
N_CORES = 8
HPC = B * H // N_CORES  # heads per core = 8
SCALE = 1.0 / (D**0.5) ** 0.5  # 1 / 64**0.25
PCHUNK = 128  # k rows per chunk
NCHUNK = S // PCHUNK  # 16
SLAB = 512  # q columns per QK matmul / AV moving tile
NSLAB = S // SLAB  # 4
BF16 = mybir.dt.bfloat16
F32 = mybir.dt.float32

_COMPILED = {}


def build_kernel():
    nc = bacc.Bacc("TRN2", target_bir_lowering=False, debug=False)
    qt = nc.dram_tensor("q_t", [HPC, D, S], BF16, kind="ExternalInput").ap()
    kt = nc.dram_tensor("k_t", [HPC, D, S], BF16, kind="ExternalInput").ap()
    v = nc.dram_tensor("v", [HPC, S, D], BF16, kind="ExternalInput").ap()
    out = nc.dram_tensor("out_t", [HPC, D, S], F32, kind="ExternalOutput").ap()
    # DRAM bounce buffers for the cross-partition softmax-denominator move
    s_dram = nc.dram_tensor("s_scratch", [HPC, S], F32).ap()
    r_dram = nc.dram_tensor("r_scratch", [HPC, S], F32).ap()

    with tile.TileContext(nc) as tc, ExitStack() as ctx:
        qk_pool = ctx.enter_context(tc.tile_pool(name="qk", bufs=2))
        v_pool = ctx.enter_context(tc.tile_pool(name="vp", bufs=2))
        exp_pool = ctx.enter_context(tc.tile_pool(name="exp", bufs=2))
        ot_pool = ctx.enter_context(tc.tile_pool(name="ot", bufs=2))
        small_pool = ctx.enter_context(tc.tile_pool(name="small", bufs=2))
        const_pool = ctx.enter_context(tc.tile_pool(name="const", bufs=1))
        psqk_pool = ctx.enter_context(
            tc.tile_pool(name="psqk", bufs=2, space="PSUM")
        )
        psav_pool = ctx.enter_context(
            tc.tile_pool(name="psav", bufs=2, space="PSUM")
        )

        zbias = const_pool.tile([128, 1], F32)
        nc.vector.memset(zbias[:], 0.0)

        TOT = HPC * NSLAB  # 32 slab iterations, pipelined across heads
        qt_sbs, kt_sbs, v_augs, ot_sbs = {}, {}, {}, {}
        prev = None  # (h, s, expT, v_aug, ot_sb) of the slab awaiting its AV

        def normalize_and_store(h, ot_sb):
            # softmax denominators: row [1, S] -> DRAM -> [128, S/128] tiles
            nc.sync.dma_start(s_dram[h], ot_sb[D : D + 1, :])
            sums_nat = small_pool.tile([128, NCHUNK], F32, tag="sums")
            nc.sync.dma_start(
                sums_nat[:], s_dram[h].rearrange("(c p) -> p c", p=128)
            )
            r_nat = small_pool.tile([128, NCHUNK], F32, tag="rnat")
            nc.vector.reciprocal(r_nat[:], sums_nat[:])
            nc.sync.dma_start(
                r_dram[h].rearrange("(c p) -> p c", p=128), r_nat[:]
            )
            r_bcast = small_pool.tile([D, S], F32, tag="rbcast")
            nc.sync.dma_start(
                r_bcast[:],
                bass.AP(r_dram.tensor, h * S, [[0, D], [1, S]]),
            )
            o_fin = ot_pool.tile([D, S], F32, tag="ofin")
            nc.vector.tensor_tensor(
                o_fin[:], ot_sb[0:D, :], r_bcast[:], op=mybir.AluOpType.mult
            )
            nc.sync.dma_start(out[h], o_fin[:])

        for t in range(TOT + 1):
            if t < TOT:
                h, s = divmod(t, NSLAB)
                if s == 0:
                    # head prologue: duplicate Q^T/K^T into partitions
                    # 64..127 so chunk pairs row-pack onto the PE (two K=64
                    # matmuls in disjoint 64-row strips of the array)
                    qt_sb = qk_pool.tile([2 * D, S], BF16, tag="qt")
                    nc.sync.dma_start(qt_sb[0:D, :], qt[h])
                    nc.sync.dma_start(qt_sb[D : 2 * D, :], qt[h])
                    kt_sb = qk_pool.tile([2 * D, S], BF16, tag="kt")
                    nc.sync.dma_start(kt_sb[0:D, :], kt[h])
                    nc.sync.dma_start(kt_sb[D : 2 * D, :], kt[h])
                    v_aug = v_pool.tile(
                        [PCHUNK, NCHUNK, D + 1], BF16, tag="vaug"
                    )
                    nc.sync.dma_start(
                        v_aug[:, :, 0:D],
                        v[h].rearrange("(c p) d -> p c d", p=PCHUNK),
                    )
                    nc.vector.memset(v_aug[:, :, D : D + 1], 1.0)
                    qt_sbs[h], kt_sbs[h], v_augs[h] = qt_sb, kt_sb, v_aug
                    # unnormalized outT + sums row accumulated across slabs
                    ot_sb = ot_pool.tile([D + 1, S], F32, tag="ot")
                    ot_sbs[h] = ot_sb
                expT = exp_pool.tile([PCHUNK, NCHUNK, SLAB], BF16, tag="expT")
            if prev is not None:
                po = psav_pool.tile([D + 1, SLAB], F32, tag="psav")

            # interleave this slab's QK/exp with the previous slab's AV so
            # the Scalar engine's exp stream never stalls at slab boundaries
            ps = None
            for c in range(NCHUNK):
                if t < TOT:
                    g, slot = divmod(c, 3)
                    if slot == 0:
                        # exp runs over 3-chunk PSUM groups (FD=1536) to
                        # amortize the per-ACTIVATE overhead; groups need not
                        # align with the row-packed matmul pairs
                        n_in = min(3, NCHUNK - c)
                        ps = psqk_pool.tile(
                            [PCHUNK, n_in, SLAB], F32, tag="psqk"
                        )
                    base = (c % 2) * D  # even chunks rows 0-63, odd 64-127
                    nc.tensor.matmul(
                        ps[:, slot, :],
                        kt_sbs[h][
                            base : base + D, c * PCHUNK : (c + 1) * PCHUNK
                        ],
                        qt_sbs[h][base : base + D, s * SLAB : (s + 1) * SLAB],
                        start=True,
                        stop=True,
                    )
                    if slot == n_in - 1:
                        nc.scalar.activation(
                            expT[:, 3 * g : 3 * g + n_in, :],
                            ps[:],
                            mybir.ActivationFunctionType.Exp,
                            bias=zbias[:],
                            scale=SCALE,
                        )
                if prev is not None:
                    # AV: V_aug stationary, expT moving -> transposed output
                    ph, psl, pexp = prev
                    nc.tensor.matmul(
                        po[:],
                        v_augs[ph][:, c, :],
                        pexp[:, c, :],
                        start=(c == 0),
                        stop=(c == NCHUNK - 1),
                    )

            if prev is not None:
                ph, psl, pexp = prev
                nc.vector.tensor_copy(
                    ot_sbs[ph][:, psl * SLAB : (psl + 1) * SLAB], po[:]
                )
                if psl == NSLAB - 1:
                    normalize_and_store(ph, ot_sbs[ph])
            prev = (h, s, expT) if t < TOT else None
    nc.compile()
    return nc


def _get_compiled():
    if "nc" not in _COMPILED:
        _COMPILED["nc"] = build_kernel()
    return _COMPILED["nc"]


def kernel(query, key, value, _want_results=False):
    nc = _get_compiled()
    q = np.asarray(query).reshape(B * H, S, D)
    k = np.asarray(key).reshape(B * H, S, D)
    v = np.asarray(value).reshape(B * H, S, D)
    in_maps = []
    for c in range(N_CORES):
        sl = slice(c * HPC, (c + 1) * HPC)
        in_maps.append(
            {
                "q_t": np.ascontiguousarray(q[sl].transpose(0, 2, 1)).astype(
                    ml_dtypes.bfloat16
                ),
                "k_t": np.ascontiguousarray(k[sl].transpose(0, 2, 1)).astype(
                    ml_dtypes.bfloat16
                ),
                "v": np.ascontiguousarray(v[sl]).astype(ml_dtypes.bfloat16),
            }
        )
    res = run_bass_kernel_spmd(nc, in_maps, core_ids=list(range(N_CORES)))
    out = np.concatenate(
        [
            res.results[c]["out_t"].transpose(0, 2, 1).reshape(1, HPC, S, D)
            for c in range(N_CORES)
        ],
        axis=0,
    ).reshape(B, H, S, D)
    if _want_results:
        return out, res
    return out


if __name__ == "__main__":
    rng = np.random.default_rng(0)
    q = rng.standard_normal((B, H, S, D), dtype=np.float32)
    k = rng.standard_normal((B, H, S, D), dtype=np.float32)
    v = rng.standard_normal((B, H, S, D), dtype=np.float32)
    o = kernel(q, k, v)
    print("kernel output", o.shape, o.dtype)
